# revision 10
# baseline (speedup 1.0000x reference)
"""Bass/Tile kernel for nn_DotAttention (batched dot-product attention).

  scores[b, t] = <hidden_decoder[b], hiddens_encoder[b, t]>
  a = softmax(scores, axis=t)
  context[b, f] = sum_t a[b, t] * hiddens_encoder[b, t, f]

Full shapes: hidden_decoder (64, 1024) f32, hiddens_encoder (64, 2048, 1024) f32,
output (64, 1024) f32.

Sharding: data-parallel over batch across 8 NeuronCores (8 batches/core),
no cross-device communication.

Per-core strategy (memory-bound: he is read from HBM exactly once; the DMA
stream runs at ~410 GB/s with the contiguous layout below):
  - t -> (partition, column) mapping is t = 16*p + c ("(p c)" layout), so a
    chunk of score columns is a fully CONTIGUOUS 16 KiB-per-partition DMA
    descriptor -- measurably faster than 4 KiB-line layouts (~166 us vs
    ~198 us for the 64 MiB stream).
  - he is cast f32 -> fp16 *during* the DMA (SWDGE cast): halves SBUF
    footprint, removes a 110 us ScalarE cast load, and enables the DVE
    2x perf mode.
  - scores, hybrid to balance DVE and ScalarE (neither has slack for all
    16 columns alone):
      cols < NTT:  DVE tensor_mul fp16 (2x mode, ~0.68 us) then ScalarE
                   Copy-activation with accum_out (~1.38 us, no 16-bit
                   accel on ScalarE).
      cols >= NTT: DVE scalar_tensor_tensor multiply+accum (1x, ~1.22 us),
                   keeping ScalarE free near the batch boundary.
  - SOFTWARE PIPELINING: each engine's queue is strictly program-ordered,
    so batch b's softmax chain (a cross-engine latency chain) is emitted
    AFTER batch b+1's score ops.  Without this, every batch boundary
    head-of-line-blocks DVE/ScalarE for ~4-6 us (measured 63 us of DVE
    idle per core).
  - softmax max: DVE free-dim max -> PE transpose -> DVE max -> -1s-matmul
    broadcast -> ACT exp with per-partition -max bias.
  - context: PE accumulating matmuls (fp16 exp-score weights, fp16 he).
    Tiny heartbeat matmuls spread through the score phase keep the PE HAM
    clock gate warm (cold PE halves matmul rate).
  - the last chunk of the last batch is split into single-column DMAs so
    the post-stream drain exposes minimal score work.
  - outputs are the unnormalized context plus Z = sum(exp); host divides.
"""

import numpy as np

import concourse.bacc as bacc
import concourse.tile as tile
from concourse import mybir
from concourse.bass_utils import run_bass_kernel_spmd

N_CORES = 8
B_FULL = 64
B = B_FULL // N_CORES  # batches per core
T = 2048
F = 1024
P = 128
NT = T // P  # 16 score columns; t = 16*p + c
COLS = 4  # score columns per he DMA chunk (2 MiB f32 source)
NCH = NT // COLS  # 4 chunks per batch
TAIL_COLS = COLS  # last chunk of last batch -> single-column DMAs
NTT = 10  # columns scored via tensor_mul+ScalarE-accum; the rest via DVE stt

F32 = mybir.dt.float32
F16 = mybir.dt.float16

_cache = {}


def _build():
    nc = bacc.Bacc("TRN2", target_bir_lowering=False, debug=False, num_devices=N_CORES)
    he = nc.dram_tensor("he", [B, T, F], F32, kind="ExternalInput").ap()
    hd = nc.dram_tensor("hd", [1, B * F], F32, kind="ExternalInput").ap()
    ident_d = nc.dram_tensor("ident", [P, P], F32, kind="ExternalInput").ap()
    out = nc.dram_tensor("out", [B, F], F32, kind="ExternalOutput").ap()
    zout = nc.dram_tensor("z", [1, B], F32, kind="ExternalOutput").ap()

    with tile.TileContext(nc) as tc:
        with (
            tc.tile_pool(name="consts", bufs=1) as consts,
            tc.tile_pool(name="hepool", bufs=15) as hepool,
            tc.tile_pool(name="tailpool", bufs=TAIL_COLS) as tailpool,
            tc.tile_pool(name="hbc", bufs=B) as hbc,
            tc.tile_pool(name="prod", bufs=8) as prodp,
            tc.tile_pool(name="asc", bufs=3) as ascp,
            tc.tile_pool(name="pdum", bufs=3) as pdump,
            tc.tile_pool(name="small", bufs=3) as small,
            tc.tile_pool(name="outp", bufs=2) as outp,
            tc.tile_pool(name="psum", bufs=2, space="PSUM") as psum_pool,
            tc.tile_pool(name="psbc", bufs=2, space="PSUM") as psbc_pool,
        ):
            neg_ones_row = consts.tile([1, P], F32)  # lhsT for -max broadcast
            nc.vector.memset(neg_ones_row[:], -1.0)
            ones_colf = consts.tile([P, 1], F32)  # rhs for the Z reduction matmul
            nc.vector.memset(ones_colf[:], 1.0)
            ident = consts.tile([P, P], F32)  # identity for PE transpose
            nc.sync.dma_start(out=ident[:], in_=ident_d[:])
            ones_row = consts.tile([1, P], F32)
            nc.vector.memset(ones_row[:], 1.0)

            # broadcast hd[b] to all 128 partitions (fp16): ones^T @ hd_row,
            # then copy/cast PSUM -> fp16 SBUF (split ACT/DVE).  Runs
            # concurrently with the first he loads (gpsimd queue).
            hdb = []
            for b in range(B):
                hd_row = small.tile([1, F], F32, tag="hdrow")
                nc.sync.dma_start(out=hd_row[:], in_=hd[0:1, b * F : (b + 1) * F])
                t_b = hbc.tile([P, F], F16)
                for j in range(2):
                    ps = psbc_pool.tile([P, 512], F32, tag="misc")
                    nc.tensor.matmul(
                        ps[:],
                        lhsT=ones_row[:],
                        rhs=hd_row[0:1, j * 512 : (j + 1) * 512],
                        start=True,
                        stop=True,
                    )
                    if j == 0:
                        nc.scalar.copy(t_b[:, j * 512 : (j + 1) * 512], ps[:])
                    else:
                        nc.vector.tensor_copy(t_b[:, j * 512 : (j + 1) * 512], ps[:])
                hdb.append(t_b)

            # issue ALL he chunk DMAs upfront (gpsimd/SWDGE queue); tile-pool
            # rotation throttles them to ~2.5 batches of prefetch
            all_hets = []
            for b in range(B):
                he_pc = he[b].rearrange("(p c) f -> p c f", p=P)  # t = 16p + c
                hets = []  # (tile, col0, ncols) in score-column order
                for k in range(NCH):
                    if b == B - 1 and k == NCH - 1:
                        for j in range(TAIL_COLS):
                            c0 = k * COLS + j
                            ht = tailpool.tile([P, 1, F], F16)
                            nc.gpsimd.dma_start(
                                out=ht[:], in_=he_pc[:, c0 : c0 + 1, :]
                            )
                            hets.append((ht, c0, 1))
                    else:
                        het = hepool.tile([P, COLS, F], F16)
                        nc.gpsimd.dma_start(
                            out=het[:], in_=he_pc[:, k * COLS : (k + 1) * COLS, :]
                        )
                        hets.append((het, k * COLS, COLS))
                all_hets.append(hets)

            def score_col(b, het, j, col, S):
                """one score column: S[:, col] = sum_f he[t, f] * hd[f]"""
                if col < NTT:
                    prod = prodp.tile([P, F], F16)
                    nc.vector.tensor_mul(prod[:], het[:, j, :], hdb[b][:])
                    asc = ascp.tile([P, F], F16)
                    nc.scalar.activation(
                        asc[:],
                        prod[:],
                        mybir.ActivationFunctionType.Copy,
                        accum_out=S[:, col : col + 1],
                    )
                else:
                    pdum = pdump.tile([P, F], F16)
                    nc.vector.scalar_tensor_tensor(
                        pdum[:],
                        het[:, j, :],
                        1.0,
                        hdb[b][:],
                        op0=mybir.AluOpType.mult,
                        op1=mybir.AluOpType.mult,
                        accum_out=S[:, col : col + 1],
                    )

            def softmax_start(prev):
                """emit max-reduce + transpose for the finished batch"""
                b, S, hets = prev
                m1 = small.tile([P, 1], F32)
                nc.vector.reduce_max(m1[:], S[:], axis=mybir.AxisListType.X)
                pst = psbc_pool.tile([1, P], F32, tag="misc")
                nc.tensor.transpose(pst[:], m1[:], ident[:])
                return pst

            def softmax_ctx(prev, pst):
                """emit the rest of softmax + context for the finished batch"""
                b, S, hets = prev
                M_sb = small.tile([1, 1], F32)
                nc.vector.reduce_max(M_sb[:], pst[:], axis=mybir.AxisListType.X)
                psb = psbc_pool.tile([P, 1], F32, tag="misc")
                nc.tensor.matmul(
                    psb[:], lhsT=neg_ones_row[:], rhs=M_sb[:], start=True, stop=True
                )
                negm = small.tile([P, 1], F32)
                nc.scalar.copy(negm[:], psb[:])
                E = small.tile([P, NT], F32)
                z1 = small.tile([P, 1], F32)
                nc.scalar.activation(
                    E[:],
                    S[:],
                    mybir.ActivationFunctionType.Exp,
                    bias=negm[:],
                    scale=1.0,
                    accum_out=z1[:],
                )
                E16 = small.tile([P, NT], F16)
                nc.scalar.copy(E16[:], E[:])

                psA = psum_pool.tile([1, 512], F32)
                psB = psum_pool.tile([1, 512], F32)
                first = True
                n_done = 0
                for het, c0, ncols in hets:
                    for j in range(ncols):
                        col = c0 + j
                        n_done += 1
                        st = first
                        sp = n_done == NT
                        first = False
                        w = E16[:, col : col + 1]
                        chunk = het[:, j, :]
                        nc.tensor.matmul(
                            psA[:], lhsT=w, rhs=chunk[:, 0:512], start=st, stop=sp
                        )
                        nc.tensor.matmul(
                            psB[:], lhsT=w, rhs=chunk[:, 512:1024], start=st, stop=sp
                        )
                psZ = psbc_pool.tile([1, 1], F32, tag="misc")
                nc.tensor.matmul(
                    psZ[:], lhsT=z1[:], rhs=ones_colf[:], start=True, stop=True
                )
                return psA, psB, psZ

            def outputs(b, psA, psB, psZ):
                ob = outp.tile([1, F], F32)
                nc.scalar.copy(ob[0:1, 0:512], psA[:])
                nc.vector.tensor_copy(ob[0:1, 512:1024], psB[:])
                zb = small.tile([1, 1], F32, tag="zb")
                nc.scalar.copy(zb[:], psZ[:])
                nc.sync.dma_start(out=out[b : b + 1, :], in_=ob[:])
                nc.sync.dma_start(out=zout[0:1, b : b + 1], in_=zb[:])

            # software-pipelined main loop: scores(b) are emitted before
            # softmax+context(b-1) finishes, so no engine queue head-of-line
            # blocks on the cross-engine softmax chain.
            prev = None  # (b, S, hets) with scores complete
            for b in range(B):
                hets = all_hets[b]
                S = small.tile([P, NT], F32, tag="S")
                flat = [
                    (het, j, c0 + j)
                    for het, c0, ncols in hets
                    for j in range(ncols)
                ]
                # first score column, bracketed by prev's max-chain pieces so
                # the cross-engine hops hide behind real DVE work
                if prev is not None:
                    pst = softmax_start(prev)
                score_col(b, flat[0][0], flat[0][1], flat[0][2], S)
                if prev is not None:
                    pA, pB, pZ = softmax_ctx(prev, pst)
                for het, j, col in flat[1:]:
                    score_col(b, het, j, col, S)
                if prev is not None:
                    outputs(prev[0], pA, pB, pZ)
                prev = (b, S, hets)

            wt = all_hets[B - 1][0][0]  # first chunk of the last batch
            for w in range(8):
                pw = psbc_pool.tile([1, 512], F32, tag="warm")
                nc.tensor.matmul(
                    pw[:],
                    lhsT=wt[:, 0, 0:1],
                    rhs=wt[:, w % 4, 0:512],
                    start=True,
                    stop=True,
                )
            pst = softmax_start(prev)
            pA, pB, pZ = softmax_ctx(prev, pst)
            outputs(prev[0], pA, pB, pZ)

    nc.compile()
    return nc


def _get_nc():
    if "nc" not in _cache:
        _cache["nc"] = _build()
    return _cache["nc"]


def _run(hidden_decoder, hiddens_encoder, trace=False, tmpdir=None):
    nc = _get_nc()
    hidden_decoder = np.ascontiguousarray(hidden_decoder, dtype=np.float32)
    hiddens_encoder = np.ascontiguousarray(hiddens_encoder, dtype=np.float32)
    ident = np.eye(P, dtype=np.float32)
    in_maps = [
        {
            "he": hiddens_encoder[i * B : (i + 1) * B],
            "hd": hidden_decoder[i * B : (i + 1) * B].reshape(1, B * F),
            "ident": ident,
        }
        for i in range(N_CORES)
    ]
    res = run_bass_kernel_spmd(
        nc, in_maps, list(range(N_CORES)), trace=trace, tmpdir=tmpdir
    )
    out = np.concatenate(
        [
            res.results[i]["out"] / res.results[i]["z"].reshape(B, 1)
            for i in range(N_CORES)
        ],
        axis=0,
    ).astype(np.float32)
    return out, res


def kernel(hidden_decoder, hiddens_encoder):
    out, _ = _run(hidden_decoder, hiddens_encoder)
    return out
